# revision 19
# baseline (speedup 1.0000x reference)
"""Bag-attention (NRE selective attention) kernel for 8 TRN2 NeuronCores, v7.

Reference computation:
    logit_i = sum_d x[i,d] * aw[q_i,d] * rw[q_i,d]
    w       = segment_softmax(logit, seg)        (bags = contiguous ranges)
    bag[b]  = sum_{i in b} w_i * x[i]
    out     = bag @ rw.T + bias

Split: the device computes the O(N*D*C) per-sentence projection
P_i = x_i @ W8 (f32 PSUM, bf16 out) over fp8-e4m3 inputs. The host
computes the O(N*D) attention logits exactly from f32 x, applies an
exact low-cost weight fix, and finishes the O(N*C) ragged softmax:
    out[b] = reduceat(e*P)/reduceat(e) + (A_b/esum_b) @ dW + bias
where dW = W - e4m3(W) (the systematic weight-quantization residual;
A_b = reduceat(e * x_hat) is an O(N*D) reduction + [B,D]@[D,C] matmul,
~3% of the device FLOPs) and e = exp(logit) (safe without
max-subtraction: logit std ~0.1).

Measured HW facts this design is built on (probed on these cores):
  - Every Matmult costs ~out_cols * 0.417ns + ~68ns fixed, regardless of
    dtype; matmul output cannot span PSUM banks (512 f32 cols), so the
    only PE lever is FEWER instructions.
  - MatmulPerfMode.DoubleRow (both operands fp8-e4/e5) contracts TWO
    128-row k-tiles per instruction at the same cost -> 690 dims = 3 DR
    matmuls per 512-col slice (pairs (0,1),(2,3),(4,tail)); stationary
    cols must be 64 (53 trips the dual-fp8 Ldweights ISA check).
  - gpsimd/SWDGE-issued input DMAs intermittently raise their completion
    semaphore before the data is visible to the PE (start-of-block
    corruption / NaN from fp8-decoding unwritten SBUF); all input DMAs
    therefore ride the HWDGE rings (sync: x + tails, scalar: weights +
    corr), whose receipt semantics proved reliable.
  - Queued DMAs progress in parallel (packet round-robin), so an unpaced
    stream completes block 0 last; depth-2 pacing on the issuing
    sequencer restores FIFO-ish completion (depth-3 measured slower).
  - Effective HBM bandwidth with all 8 cores streaming is ~255-265
    GB/s/core (not the 358 single-core limit), so the kernel is
    DMA-byte-bound: ~5.5us NEFF startup + bytes/BW + ~4us drain.
  - The tail chunk (dims 640:690) is padded to 64 DMA partitions: 64
    splits over all 16 SDMA engines (airtight semaphore accounting) and
    the leftover [64:128) memset starts on a legal partition boundary
    and never overlaps the DMA. fp8 NaN bytes x zero-W = NaN, so those
    rows must hold finite values.

Quantization (validated vs reference in f64; HW matches sim exactly,
rel 9.4e-3 < 2e-2):
  - x ships as e4m3 (11.6 MB/core vs 18.4 baseline); per-sentence fp8
    noise averages out inside large bags, and the ~4K sentences in the
    globally smallest bags ship an extra e5m2 residual x - e4m3(x)
    whose bf16 P-corrections the host adds back by index (one 512-col
    block per core, filled round-robin so cores stay balanced).
  - W ships as a single e4m3 pass; its systematic error is removed by
    the exact host dW term above.

Block plan: correction block first (small, loads early, warms the PE),
then 7x2048 + 2x1024 main blocks - the small final blocks shorten the
pipe tail after the last input byte. Per-core traffic ~13.9 MB.

Sharding: 16384 contiguous sentences per core; weights replicated; the
ragged segment softmax runs on the host so bags straddling core
boundaries need no special handling.
"""

import sys

_REPO = "/opt/trn_rl_repo"
if _REPO not in sys.path:
    sys.path.insert(0, _REPO)

import numpy as np
import ml_dtypes

N_SENT = 131072
REL_DIM = 690
C = 53
NCORES = 8
NS = N_SENT // NCORES  # sentences per core
PMAIN = 128
NCHM = 5               # main chunks of 128 dims (0:640)
TAIL = 50              # tail dims 640:690
TPAD = 64              # tail DMA partitions (50 dims + 14 zero rows;
                       # 64 -> all 16 SDMA engines carry data so the
                       # completion semaphore is airtight, and the
                       # remaining memset [64:128] starts on a legal
                       # partition boundary and never overlaps the DMA)
NPAIR = 3              # DR pairs: (0,1), (2,3), (4, tail zero-padded)
BLK = 2048             # sentences per compute block
SUB = 512              # matmul col slice (one PSUM bank = 512 f32)
NBLK = NS // BLK       # 8 main blocks; block NBLK is the correction block
MM = 64                # stationary cols / PSUM partitions (53 + 11 zero
                       # pad; the dual-fp8 Ldweights encoding rejects 53)
OROWS = 56             # output rows (53 + 3 pad; 56 -> 14 DMA engines)
CBLK = 512             # correction block cols
CORR_CAP = NCORES * CBLK

_NC_CACHE = {}


def _build():
    import concourse.bass as bass
    from concourse import mybir

    f32 = mybir.dt.float32
    bf16 = mybir.dt.bfloat16
    e4 = mybir.dt.float8e4
    e5 = mybir.dt.float8e5
    DR = mybir.MatmulPerfMode.DoubleRow

    nc = bass.Bass()
    # main transfer plan: 7 x 2048 + 2 x 1024 (small final blocks shorten
    # the pipe tail after the last input byte lands)
    XCOLS = [BLK] * 7 + [BLK // 2, BLK // 4, BLK // 4]
    XOFF = [sum(XCOLS[:i]) for i in range(len(XCOLS))]
    NXB = len(XCOLS)

    xm = nc.declare_dram_parameter("xm", [PMAIN, NBLK, NCHM, BLK], e4, isOutput=False)
    xt = nc.declare_dram_parameter("xt", [TPAD, NBLK, BLK], e4, isOutput=False)
    cm = nc.declare_dram_parameter("cm", [PMAIN, NCHM, CBLK], e5, isOutput=False)
    ct = nc.declare_dram_parameter("ct", [TPAD, CBLK], e5, isOutput=False)
    w8 = nc.declare_dram_parameter("w8", [PMAIN, NPAIR * 2 * MM], e4, isOutput=False)
    out = nc.declare_dram_parameter("out", [OROWS, CBLK + NS], bf16,
                                    isOutput=True)

    from contextlib import ExitStack
    with ExitStack() as stk:
        # 6 chunks of 128 dims; chunk 5 = dims 640:690 in rows 0:64 (50
        # real + 14 shipped zeros), rows 64:128 zeroed by memset once
        xbuf = stk.enter_context(nc.sbuf_tensor("xbuf", [PMAIN, NBLK, 6, BLK], e4))
        cbuf = stk.enter_context(nc.sbuf_tensor("cbuf", [PMAIN, 6, CBLK], e5))
        w8_sb = stk.enter_context(nc.sbuf_tensor("w8_sb", [PMAIN, NPAIR, 2, MM], e4))
        out_sb = stk.enter_context(nc.sbuf_tensor("out_sb", [OROWS, 2, BLK], bf16))
        psb = [stk.enter_context(nc.psum_tensor(f"ps{i}", [MM, BLK], f32))
               for i in range(2)]

        s_x = [stk.enter_context(nc.semaphore(f"s_x{i}")) for i in range(NXB)]
        s_xt = stk.enter_context(nc.semaphore("s_xt"))
        s_c = stk.enter_context(nc.semaphore("s_c"))
        s_w = stk.enter_context(nc.semaphore("s_w"))
        s_z = stk.enter_context(nc.semaphore("s_z"))
        s_mm = stk.enter_context(nc.semaphore("s_mm"))
        s_cpa = stk.enter_context(nc.semaphore("s_cpa"))
        s_cpv = stk.enter_context(nc.semaphore("s_cpv"))
        s_od = stk.enter_context(nc.semaphore("s_od"))
        block = stk.enter_context(nc.Block())

        # PE block plan: corr first (its inputs are small and load first,
        # warming the PE while main block 0 streams), then main blocks

        @block.sync
        def _(sync):
            ntail = 0
            for k in range(NXB):
                # depth-2 pacing: completion order ~ issue order, so early
                # blocks land early; deeper pacing measured slower
                if k >= 2:
                    sync.wait_ge(s_x[k - 2], 16)
                off = XOFF[k]
                db, o = off // BLK, off % BLK
                sync.dma_start(
                    out=xbuf[:, db, 0:NCHM, o:o + XCOLS[k]],
                    in_=xm[:, db, :, o:o + XCOLS[k]],
                ).then_inc(s_x[k], 16)
                if o == 0:
                    # tail chunk for this dblock rides the same HWDGE ring
                    sync.dma_start(
                        out=xbuf[0:TPAD, db, NCHM, :], in_=xt[:, db, :],
                    ).then_inc(s_xt, 16)
                    ntail += 1
            assert ntail == NBLK

        @block.tensor
        def _(pe):
            pe.wait_ge(s_w, 16)
            for k in range(NXB + 1):
                i = k % 2
                corr = k == 0
                ncols = CBLK if corr else XCOLS[k - 1]
                if corr:
                    pe.wait_ge(s_z, 1)
                    pe.wait_ge(s_c, 32)
                else:
                    off = XOFF[k - 1]
                    db_lo = off // BLK
                    db_hi = (off + ncols - 1) // BLK
                    pe.wait_ge(s_z, db_hi + 2)
                    pe.wait_ge(s_x[k - 1], 16)
                    pe.wait_ge(s_xt, 16 * (db_hi + 1))
                if k >= 2:
                    # copy(k-2) freed ps[i]; even blocks copy on ACT, odd on DVE
                    if k % 2 == 0:
                        pe.wait_ge(s_cpa, k // 2)
                    else:
                        pe.wait_ge(s_cpv, (k - 1) // 2)
                for sub in range(ncols // SUB):
                    s0, s1 = sub * SUB, (sub + 1) * SUB
                    ps = psb[i][:, s0:s1]
                    for p in range(NPAIR):
                        if corr:
                            mv = cbuf[:, 2 * p:2 * p + 2, s0:s1]
                        else:
                            off = XOFF[k - 1]
                            db, o = off // BLK, off % BLK
                            mv = xbuf[:, db, 2 * p:2 * p + 2, o + s0:o + s1]
                        mmt = nc.tensor.matmul(
                            ps, w8_sb[:, p, :, :], mv,
                            start=(p == 0), stop=(p == NPAIR - 1),
                            perf_mode=DR,
                        )
                mmt.then_inc(s_mm, 1)

        @block.vector
        def _(dve):
            nc.vector.memset(cbuf[TPAD:PMAIN, NCHM, :], 0.0).then_inc(s_z, 1)
            for db in range(NBLK):
                nc.vector.memset(xbuf[TPAD:PMAIN, db, NCHM, :], 0.0).then_inc(s_z, 1)
            nc.vector.memset(out_sb[:, :, :], 0.0)
            for k in range(1, NXB + 1, 2):
                i = k % 2
                ncols = XCOLS[k - 1]
                dve.wait_ge(s_mm, k + 1)
                if k >= 2:
                    dve.wait_ge(s_od, 16 * (k - 1))  # out-DMA(k-2) freed out_sb[i]
                nc.vector.tensor_copy(
                    out_sb[0:C, i, 0:ncols], psb[i][0:C, 0:ncols]
                ).then_inc(s_cpv, 1)

        @block.scalar
        def _(act):
            nc.scalar.dma_start(out=w8_sb[:], in_=w8[:]).then_inc(s_w, 16)
            nc.scalar.dma_start(out=cbuf[:, 0:NCHM, :], in_=cm[:]).then_inc(s_c, 16)
            nc.scalar.dma_start(out=cbuf[0:TPAD, NCHM, :], in_=ct[:]).then_inc(s_c, 16)
            ocol = 0
            for k in range(NXB + 1):
                i = k % 2
                ncols = CBLK if k == 0 else XCOLS[k - 1]
                if k % 2 == 0:
                    act.wait_ge(s_mm, k + 1)
                    if k >= 2:
                        act.wait_ge(s_od, 16 * (k - 1))  # out_sb[i] free
                    nc.scalar.copy(
                        out_sb[0:C, i, 0:ncols], psb[i][0:C, 0:ncols]
                    ).then_inc(s_cpa, 1)
                    act.wait_ge(s_cpa, k // 2 + 1)
                else:
                    act.wait_ge(s_cpv, (k + 1) // 2)
                act.dma_start(
                    out=out[:, ocol:ocol + ncols],
                    in_=out_sb[:, i, 0:ncols],
                ).then_inc(s_od, 16)
                ocol += ncols

    return nc


def _get_nc():
    if "nc" not in _NC_CACHE:
        _NC_CACHE["nc"] = _build()
    return _NC_CACHE["nc"]


def _pack_x(xq8, tail_nblk, tail_blk):
    """[ns, 690] fp8 -> main [128, nblk, 5, blk] (contiguous 10KB per
    partition per dblock) + tail [64, tail_nblk, tail_blk] (rows 50:64
    zero)."""
    t = np.ascontiguousarray(xq8.T)  # [690, ns]
    nblk = max(1, xq8.shape[0] // BLK)
    blk = xq8.shape[0] // nblk
    main = t[0:PMAIN * NCHM].reshape(NCHM, PMAIN, nblk, blk).transpose(1, 2, 0, 3)
    tail = np.zeros((TPAD, tail_nblk, tail_blk), dtype=xq8.dtype)
    tail[0:TAIL] = t[PMAIN * NCHM:].reshape(TAIL, tail_nblk, tail_blk)
    return np.ascontiguousarray(main), tail


def _prepare(x, relation_weight, scope):
    e4m3 = ml_dtypes.float8_e4m3
    e5m2 = ml_dtypes.float8_e5m2
    x = np.asarray(x, dtype=np.float32)
    rw = np.asarray(relation_weight, dtype=np.float32)

    wmat = np.zeros((PMAIN * 6, MM), dtype=np.float32)
    wmat[0:REL_DIM, 0:C] = rw.T  # zero pad: cols 53:64, dim rows 690:768
    w8f = wmat.astype(e4m3)
    dW = (wmat - w8f.astype(np.float32))[0:REL_DIM, 0:C]  # [690, 53]
    # [768, 64] -> [128, pair, 2, 64]: row r, pair p, k-tile k = dim
    # p*256 + k*128 + r
    w8_p = np.ascontiguousarray(
        w8f.reshape(NPAIR, 2, PMAIN, MM).transpose(2, 0, 1, 3)).reshape(PMAIN, -1)

    x8 = x.astype(e4m3)

    # correction set: sentences in the globally smallest bags, capped
    scope = np.asarray(scope).astype(np.int64)
    sizes = np.diff(scope)
    seg = np.repeat(np.arange(sizes.shape[0]), sizes)
    ssz = sizes[seg]                          # bag size per sentence
    order = np.argsort(ssz, kind="stable")
    ncorr = int(min(CORR_CAP, int((ssz <= 16).sum())))
    corr_idx = order[:ncorr]

    r_all = np.zeros((CORR_CAP, REL_DIM), dtype=e5m2)
    r_all[:ncorr] = (x[corr_idx] - x8[corr_idx].astype(np.float32)).astype(e5m2)

    in_maps = []
    for m in range(NCORES):
        sl = slice(m * NS, (m + 1) * NS)
        xm_p, xt_p = _pack_x(x8[sl], NBLK, BLK)
        cm_p, ct_p = _pack_x(r_all[m * CBLK:(m + 1) * CBLK], 1, CBLK)
        in_maps.append({
            "xm": xm_p, "xt": xt_p,
            "cm": np.ascontiguousarray(cm_p[:, 0]),
            "ct": np.ascontiguousarray(ct_p[:, 0]),
            "w8": w8_p,
        })
    return in_maps, corr_idx, x8, r_all, dW


def _finish(P, x, x8, r_all, corr_idx, dW, aw, rw, attention_query, scope,
            bias):
    m = (aw * rw).astype(np.float32)  # [53, 690]
    q = np.asarray(attention_query).astype(np.int64)
    scope = np.asarray(scope).astype(np.int64)
    logit = np.empty(N_SENT, dtype=np.float32)
    step = 16384
    for i in range(0, N_SENT, step):
        logit[i:i + step] = np.einsum(
            "nd,nd->n", x[i:i + step], m[q[i:i + step]], optimize=True)
    e = np.exp(logit.astype(np.float64))
    esum = np.add.reduceat(e, scope[:-1])

    # exact fix of the systematic W-quantization term: (sum_bag e*x_hat)@dW
    ncorr = corr_idx.shape[0]
    xh = x8.astype(np.float32)
    xh[corr_idx[:ncorr]] += r_all[:ncorr].astype(np.float32)
    A = np.empty((scope.shape[0] - 1, REL_DIM), dtype=np.float64)
    ew = e[:, None]
    # chunked weighted reduceat to bound temp memory
    bstart = scope[:-1]
    A[:] = np.add.reduceat(xh * ew.astype(np.float32), bstart, axis=0)
    fix = (A / esum[:, None]) @ dW.astype(np.float64)

    sums = np.add.reduceat(P * e[:, None], scope[:-1], axis=0)
    logits = sums / esum[:, None] + fix + np.asarray(bias, np.float64)[None, :]
    return logits.astype(np.float32)


def _run(inputs, trace=False, **kw):
    from concourse.bass_utils import run_bass_kernel_spmd

    nc = _get_nc()
    x = np.asarray(inputs["x"], dtype=np.float32)
    in_maps, corr_idx, x8, r_all, dW = _prepare(
        x, inputs["relation_weight"], inputs["scope"])
    res = run_bass_kernel_spmd(nc, in_maps, core_ids=list(range(NCORES)),
                               trace=trace, **kw)
    outs = np.stack([np.asarray(r["out"]) for r in res.results])
    GP = np.asarray(outs, dtype=np.float32)
    P = GP[:, 0:C, CBLK:].transpose(0, 2, 1).reshape(N_SENT, C).astype(np.float64)
    Pc = GP[:, 0:C, 0:CBLK].transpose(0, 2, 1).reshape(CORR_CAP, C)
    ncorr = corr_idx.shape[0]
    P[corr_idx] += Pc[:ncorr].astype(np.float64)
    logits = _finish(
        P, x, x8, r_all, corr_idx, dW,
        np.asarray(inputs["attention_weight"], dtype=np.float32),
        np.asarray(inputs["relation_weight"], dtype=np.float32),
        inputs["attention_query"], inputs["scope"],
        np.asarray(inputs["bias"], np.float32))
    return logits, res


def kernel(x, relation_weight, attention_weight, bias, attention_query, scope):
    logits, _ = _run(dict(x=x, relation_weight=relation_weight,
                          attention_weight=attention_weight, bias=bias,
                          attention_query=attention_query, scope=scope))
    return logits


# revision 21
# speedup vs baseline: 1.1224x; 1.1224x over previous
"""Bag-attention (NRE selective attention) kernel for 8 TRN2 NeuronCores, v7.

Reference computation:
    logit_i = sum_d x[i,d] * aw[q_i,d] * rw[q_i,d]
    w       = segment_softmax(logit, seg)        (bags = contiguous ranges)
    bag[b]  = sum_{i in b} w_i * x[i]
    out     = bag @ rw.T + bias

Split: the device computes the O(N*D*C) per-sentence projection
P_i = x_i @ W8 (f32 PSUM, bf16 out) over fp8-e4m3 inputs. The host
computes the O(N*D) attention logits exactly from f32 x, applies an
exact low-cost weight fix, and finishes the O(N*C) ragged softmax:
    out[b] = reduceat(e*P)/reduceat(e) + (A_b/esum_b) @ dW + bias
where dW = W - e4m3(W) (the systematic weight-quantization residual;
A_b = reduceat(e * x_hat) is an O(N*D) reduction + [B,D]@[D,C] matmul,
~3% of the device FLOPs) and e = exp(logit) (safe without
max-subtraction: logit std ~0.1).

Measured HW facts this design is built on (probed on these cores):
  - Every Matmult costs ~out_cols * 0.417ns + ~68ns fixed, regardless of
    dtype; matmul output cannot span PSUM banks (512 f32 cols), so the
    only PE lever is FEWER instructions.
  - MatmulPerfMode.DoubleRow (both operands fp8-e4/e5) contracts TWO
    128-row k-tiles per instruction at the same cost -> 690 dims = 3 DR
    matmuls per 512-col slice (pairs (0,1),(2,3),(4,tail)); stationary
    cols must be 64 (53 trips the dual-fp8 Ldweights ISA check).
  - gpsimd/SWDGE-issued input DMAs intermittently raise their completion
    semaphore before the data is visible to the PE (start-of-block
    corruption / NaN from fp8-decoding unwritten SBUF); all input DMAs
    therefore ride the HWDGE rings (sync: x + tails, scalar: weights +
    corr), whose receipt semantics proved reliable.
  - Queued DMAs progress in parallel (packet round-robin), so an unpaced
    stream completes block 0 last; depth-2 pacing on the issuing
    sequencer restores FIFO-ish completion (depth-3 measured slower).
  - Effective HBM bandwidth with all 8 cores streaming is ~255-265
    GB/s/core (not the 358 single-core limit), so the kernel is
    DMA-byte-bound: ~5.5us NEFF startup + bytes/BW + ~4us drain.
  - The tail chunk (dims 640:690) is padded to 64 DMA partitions: 64
    splits over all 16 SDMA engines (airtight semaphore accounting) and
    the leftover [64:128) memset starts on a legal partition boundary
    and never overlaps the DMA. fp8 NaN bytes x zero-W = NaN, so those
    rows must hold finite values.

Quantization (validated vs reference in f64; HW matches sim exactly,
rel 9.4e-3 < 2e-2):
  - x ships as e4m3 (11.6 MB/core vs 18.4 baseline); per-sentence fp8
    noise averages out inside large bags, and the ~4K sentences in the
    globally smallest bags ship an extra e5m2 residual x - e4m3(x)
    whose bf16 P-corrections the host adds back by index (one 512-col
    block per core, filled round-robin so cores stay balanced).
  - W ships as a single e4m3 pass; its systematic error is removed by
    the exact host dW term above.

Block plan: correction block first (small, loads early, warms the PE),
then 7x2048 + 2x1024 main blocks - the small final blocks shorten the
pipe tail after the last input byte. Per-core traffic ~13.9 MB.

Sharding: 16384 contiguous sentences per core; weights replicated; the
ragged segment softmax runs on the host so bags straddling core
boundaries need no special handling.
"""

import sys

_REPO = "/opt/trn_rl_repo"
if _REPO not in sys.path:
    sys.path.insert(0, _REPO)

import numpy as np
import ml_dtypes

N_SENT = 131072
REL_DIM = 690
C = 53
NCORES = 8
NS = N_SENT // NCORES  # sentences per core
PMAIN = 128
NCHM = 5               # main chunks of 128 dims (0:640)
TAIL = 50              # tail dims 640:690
TPAD = 64              # tail DMA partitions (50 dims + 14 zero rows;
                       # 64 -> all 16 SDMA engines carry data so the
                       # completion semaphore is airtight, and the
                       # remaining memset [64:128] starts on a legal
                       # partition boundary and never overlaps the DMA)
NPAIR = 3              # DR pairs: (0,1), (2,3), (4, tail zero-padded)
BLK = 2048             # sentences per compute block
SUB = 512              # matmul col slice (one PSUM bank = 512 f32)
NBLK = NS // BLK       # 8 main blocks; block NBLK is the correction block
MM = 64                # stationary cols / PSUM partitions (53 + 11 zero
                       # pad; the dual-fp8 Ldweights encoding rejects 53)
OROWS = 56             # output rows (53 + 3 pad; 56 -> 14 DMA engines)
CBLK = 512             # correction block cols
CORR_CAP = NCORES * CBLK

_NC_CACHE = {}


def _build():
    import concourse.bass as bass
    from concourse import mybir

    f32 = mybir.dt.float32
    bf16 = mybir.dt.bfloat16
    e4 = mybir.dt.float8e4
    e5 = mybir.dt.float8e5
    DR = mybir.MatmulPerfMode.DoubleRow

    nc = bass.Bass()
    # main transfer plan: 7 x 2048 + 1024 + 2 x 512 (small final blocks
    # shorten the pipe tail after the last input byte lands: final PE
    # block, its copy, and its out-DMA are all quarter-size)
    XCOLS = [BLK] * 7 + [BLK // 2, BLK // 4, BLK // 4]
    XOFF = [sum(XCOLS[:i]) for i in range(len(XCOLS))]
    NXB = len(XCOLS)

    xm = nc.declare_dram_parameter("xm", [PMAIN, NBLK, NCHM, BLK], e4, isOutput=False)
    xt = nc.declare_dram_parameter("xt", [TPAD, NBLK, BLK], e4, isOutput=False)
    cm = nc.declare_dram_parameter("cm", [PMAIN, NCHM, CBLK], e5, isOutput=False)
    ct = nc.declare_dram_parameter("ct", [TPAD, CBLK], e5, isOutput=False)
    w8 = nc.declare_dram_parameter("w8", [PMAIN, NPAIR * 2 * MM], e4, isOutput=False)
    out = nc.declare_dram_parameter("out", [OROWS, CBLK + NS], bf16,
                                    isOutput=True)

    from contextlib import ExitStack
    with ExitStack() as stk:
        # 6 chunks of 128 dims; chunk 5 = dims 640:690 in rows 0:64 (50
        # real + 14 shipped zeros), rows 64:128 zeroed by memset once
        xbuf = stk.enter_context(nc.sbuf_tensor("xbuf", [PMAIN, NBLK, 6, BLK], e4))
        cbuf = stk.enter_context(nc.sbuf_tensor("cbuf", [PMAIN, 6, CBLK], e5))
        w8_sb = stk.enter_context(nc.sbuf_tensor("w8_sb", [PMAIN, NPAIR, 2, MM], e4))
        out_sb = stk.enter_context(nc.sbuf_tensor("out_sb", [OROWS, 2, BLK], bf16))
        psb = [stk.enter_context(nc.psum_tensor(f"ps{i}", [MM, BLK], f32))
               for i in range(2)]

        s_x = [stk.enter_context(nc.semaphore(f"s_x{i}")) for i in range(NXB)]
        s_xt = stk.enter_context(nc.semaphore("s_xt"))
        s_c = stk.enter_context(nc.semaphore("s_c"))
        s_w = stk.enter_context(nc.semaphore("s_w"))
        s_z = stk.enter_context(nc.semaphore("s_z"))
        s_mm = stk.enter_context(nc.semaphore("s_mm"))
        s_cp = stk.enter_context(nc.semaphore("s_cp"))
        s_od = stk.enter_context(nc.semaphore("s_od"))
        block = stk.enter_context(nc.Block())

        # PE block plan: corr first (its inputs are small and load first,
        # warming the PE while main block 0 streams), then main blocks

        @block.sync
        def _(sync):
            ntail = 0
            for k in range(NXB):
                # depth-2 pacing: completion order ~ issue order, so early
                # blocks land early; deeper pacing measured slower
                if k >= 2:
                    sync.wait_ge(s_x[k - 2], 16)
                off = XOFF[k]
                db, o = off // BLK, off % BLK
                sync.dma_start(
                    out=xbuf[:, db, 0:NCHM, o:o + XCOLS[k]],
                    in_=xm[:, db, :, o:o + XCOLS[k]],
                ).then_inc(s_x[k], 16)
                if o == 0:
                    # tail chunk for this dblock rides the same HWDGE ring
                    sync.dma_start(
                        out=xbuf[0:TPAD, db, NCHM, :], in_=xt[:, db, :],
                    ).then_inc(s_xt, 16)
                    ntail += 1
            assert ntail == NBLK

        @block.tensor
        def _(pe):
            pe.wait_ge(s_w, 16)
            for k in range(NXB + 1):
                i = k % 2
                corr = k == 0
                ncols = CBLK if corr else XCOLS[k - 1]
                if corr:
                    pe.wait_ge(s_z, 1)
                    pe.wait_ge(s_c, 32)
                else:
                    off = XOFF[k - 1]
                    db_lo = off // BLK
                    db_hi = (off + ncols - 1) // BLK
                    pe.wait_ge(s_z, db_hi + 2)
                    pe.wait_ge(s_x[k - 1], 16)
                    pe.wait_ge(s_xt, 16 * (db_hi + 1))
                if k >= 2:
                    pe.wait_ge(s_cp, k - 1)  # copy(k-2) freed ps[i]
                for sub in range(ncols // SUB):
                    s0, s1 = sub * SUB, (sub + 1) * SUB
                    ps = psb[i][:, s0:s1]
                    for p in range(NPAIR):
                        if corr:
                            mv = cbuf[:, 2 * p:2 * p + 2, s0:s1]
                        else:
                            off = XOFF[k - 1]
                            db, o = off // BLK, off % BLK
                            mv = xbuf[:, db, 2 * p:2 * p + 2, o + s0:o + s1]
                        mmt = nc.tensor.matmul(
                            ps, w8_sb[:, p, :, :], mv,
                            start=(p == 0), stop=(p == NPAIR - 1),
                            perf_mode=DR,
                        )
                mmt.then_inc(s_mm, 1)

        @block.vector
        def _(dve):
            nc.vector.memset(cbuf[TPAD:PMAIN, NCHM, :], 0.0).then_inc(s_z, 1)
            for db in range(NBLK):
                nc.vector.memset(xbuf[TPAD:PMAIN, db, NCHM, :], 0.0).then_inc(s_z, 1)
            nc.vector.memset(out_sb[:, :, :], 0.0)
            for k in range(NXB + 1):
                i = k % 2
                ncols = CBLK if k == 0 else XCOLS[k - 1]
                dve.wait_ge(s_mm, k + 1)
                if k >= 2:
                    dve.wait_ge(s_od, 16 * (k - 1))  # out-DMA(k-2) freed out_sb[i]
                nc.vector.tensor_copy(
                    out_sb[0:C, i, 0:ncols], psb[i][0:C, 0:ncols]
                ).then_inc(s_cp, 1)

        @block.scalar
        def _(act):
            nc.scalar.dma_start(out=w8_sb[:], in_=w8[:]).then_inc(s_w, 16)
            nc.scalar.dma_start(out=cbuf[:, 0:NCHM, :], in_=cm[:]).then_inc(s_c, 16)
            nc.scalar.dma_start(out=cbuf[0:TPAD, NCHM, :], in_=ct[:]).then_inc(s_c, 16)
            ocol = 0
            for k in range(NXB + 1):
                i = k % 2
                ncols = CBLK if k == 0 else XCOLS[k - 1]
                act.wait_ge(s_cp, k + 1)
                act.dma_start(
                    out=out[:, ocol:ocol + ncols],
                    in_=out_sb[:, i, 0:ncols],
                ).then_inc(s_od, 16)
                ocol += ncols

    return nc


def _get_nc():
    if "nc" not in _NC_CACHE:
        _NC_CACHE["nc"] = _build()
    return _NC_CACHE["nc"]


def _pack_x(xq8, tail_nblk, tail_blk):
    """[ns, 690] fp8 -> main [128, nblk, 5, blk] (contiguous 10KB per
    partition per dblock) + tail [64, tail_nblk, tail_blk] (rows 50:64
    zero)."""
    t = np.ascontiguousarray(xq8.T)  # [690, ns]
    nblk = max(1, xq8.shape[0] // BLK)
    blk = xq8.shape[0] // nblk
    main = t[0:PMAIN * NCHM].reshape(NCHM, PMAIN, nblk, blk).transpose(1, 2, 0, 3)
    tail = np.zeros((TPAD, tail_nblk, tail_blk), dtype=xq8.dtype)
    tail[0:TAIL] = t[PMAIN * NCHM:].reshape(TAIL, tail_nblk, tail_blk)
    return np.ascontiguousarray(main), tail


def _prepare(x, relation_weight, scope):
    e4m3 = ml_dtypes.float8_e4m3
    e5m2 = ml_dtypes.float8_e5m2
    x = np.asarray(x, dtype=np.float32)
    rw = np.asarray(relation_weight, dtype=np.float32)

    wmat = np.zeros((PMAIN * 6, MM), dtype=np.float32)
    wmat[0:REL_DIM, 0:C] = rw.T  # zero pad: cols 53:64, dim rows 690:768
    w8f = wmat.astype(e4m3)
    dW = (wmat - w8f.astype(np.float32))[0:REL_DIM, 0:C]  # [690, 53]
    # [768, 64] -> [128, pair, 2, 64]: row r, pair p, k-tile k = dim
    # p*256 + k*128 + r
    w8_p = np.ascontiguousarray(
        w8f.reshape(NPAIR, 2, PMAIN, MM).transpose(2, 0, 1, 3)).reshape(PMAIN, -1)

    x8 = x.astype(e4m3)

    # correction set: sentences in the globally smallest bags, capped
    scope = np.asarray(scope).astype(np.int64)
    sizes = np.diff(scope)
    seg = np.repeat(np.arange(sizes.shape[0]), sizes)
    ssz = sizes[seg]                          # bag size per sentence
    order = np.argsort(ssz, kind="stable")
    ncorr = int(min(CORR_CAP, int((ssz <= 16).sum())))
    corr_idx = order[:ncorr]

    r_all = np.zeros((CORR_CAP, REL_DIM), dtype=e5m2)
    r_all[:ncorr] = (x[corr_idx] - x8[corr_idx].astype(np.float32)).astype(e5m2)

    in_maps = []
    for m in range(NCORES):
        sl = slice(m * NS, (m + 1) * NS)
        xm_p, xt_p = _pack_x(x8[sl], NBLK, BLK)
        cm_p, ct_p = _pack_x(r_all[m * CBLK:(m + 1) * CBLK], 1, CBLK)
        in_maps.append({
            "xm": xm_p, "xt": xt_p,
            "cm": np.ascontiguousarray(cm_p[:, 0]),
            "ct": np.ascontiguousarray(ct_p[:, 0]),
            "w8": w8_p,
        })
    return in_maps, corr_idx, x8, r_all, dW


def _finish(P, x, x8, r_all, corr_idx, dW, aw, rw, attention_query, scope,
            bias):
    m = (aw * rw).astype(np.float32)  # [53, 690]
    q = np.asarray(attention_query).astype(np.int64)
    scope = np.asarray(scope).astype(np.int64)
    logit = np.empty(N_SENT, dtype=np.float32)
    step = 16384
    for i in range(0, N_SENT, step):
        logit[i:i + step] = np.einsum(
            "nd,nd->n", x[i:i + step], m[q[i:i + step]], optimize=True)
    e = np.exp(logit.astype(np.float64))
    esum = np.add.reduceat(e, scope[:-1])

    # exact fix of the systematic W-quantization term: (sum_bag e*x_hat)@dW
    ncorr = corr_idx.shape[0]
    xh = x8.astype(np.float32)
    xh[corr_idx[:ncorr]] += r_all[:ncorr].astype(np.float32)
    A = np.empty((scope.shape[0] - 1, REL_DIM), dtype=np.float64)
    ew = e[:, None]
    # chunked weighted reduceat to bound temp memory
    bstart = scope[:-1]
    A[:] = np.add.reduceat(xh * ew.astype(np.float32), bstart, axis=0)
    fix = (A / esum[:, None]) @ dW.astype(np.float64)

    sums = np.add.reduceat(P * e[:, None], scope[:-1], axis=0)
    logits = sums / esum[:, None] + fix + np.asarray(bias, np.float64)[None, :]
    return logits.astype(np.float32)


def _run(inputs, trace=False, **kw):
    from concourse.bass_utils import run_bass_kernel_spmd

    nc = _get_nc()
    x = np.asarray(inputs["x"], dtype=np.float32)
    in_maps, corr_idx, x8, r_all, dW = _prepare(
        x, inputs["relation_weight"], inputs["scope"])
    res = run_bass_kernel_spmd(nc, in_maps, core_ids=list(range(NCORES)),
                               trace=trace, **kw)
    outs = np.stack([np.asarray(r["out"]) for r in res.results])
    GP = np.asarray(outs, dtype=np.float32)
    P = GP[:, 0:C, CBLK:].transpose(0, 2, 1).reshape(N_SENT, C).astype(np.float64)
    Pc = GP[:, 0:C, 0:CBLK].transpose(0, 2, 1).reshape(CORR_CAP, C)
    ncorr = corr_idx.shape[0]
    P[corr_idx] += Pc[:ncorr].astype(np.float64)
    logits = _finish(
        P, x, x8, r_all, corr_idx, dW,
        np.asarray(inputs["attention_weight"], dtype=np.float32),
        np.asarray(inputs["relation_weight"], dtype=np.float32),
        inputs["attention_query"], inputs["scope"],
        np.asarray(inputs["bias"], np.float32))
    return logits, res


def kernel(x, relation_weight, attention_weight, bias, attention_query, scope):
    logits, _ = _run(dict(x=x, relation_weight=relation_weight,
                          attention_weight=attention_weight, bias=bias,
                          attention_query=attention_query, scope=scope))
    return logits
